# revision 23
# baseline (speedup 1.0000x reference)
"""Trainium2 Bass kernel for nn_AttentionDist (attention + per-class logsumexp).

Math (per batch b):
    logits = queries @ context.T * mask          [Q, K]
    p      = softmax(logits, axis=-1)
    cv     = p @ context                          [Q, D]
    lsc[q,c] = logsumexp_{k: labels[k]==c} logp[q,k] = ln(S_c[q] / T[q])
  where P = exp(logits)  (no max-subtraction needed: |logits| <~ 50 fits f32),
        S_c = sum_{k in class c} P[q,k],  T = sum_k P[q,k].

Kernel strategy (data-parallel, core i <- batch i):
  MM1 (f16):   logitsT[kt]  = cT[:,kt].T @ qT          -> PSUM [128k, 1024q]
               even k-tiles use PE rows 0:64, odd k-tiles rows 64:128
               (operands duplicated across both partition halves) so
               consecutive LDWEIGHTS/MATMULs land on different row groups
               and overlap in the PE array.
  ACT:         P^T[kt]      = exp(logitsT[kt])         -> SBUF bf16
  MM2 (bf16):  acc         += [ctx|1|onehot|0][kt].T @ P^T[kt] -> PSUM [128, 1024]
               rows 0:64 = cv.T unnorm, 64 = T, 65:86 = S.T, 86:128 zeros
  Finale: PE-transpose acc to [q, 86] tiles, divide by T, ln for lsc.

The MM1->exp->MM2 chain is software-pipelined (MM2 trails by SKEW k-tiles)
so the PE never head-of-line blocks on the exp of the current k-tile. The
mask is folded in on the CPU by scaling cT columns.
"""

from contextlib import ExitStack

import numpy as np
import ml_dtypes

B, Q, K, D = 8, 1024, 4096, 64
C1 = 21
KT = K // 128   # 32 k-tiles
QT = Q // 128   # 8 q-tiles
W = D + 1 + C1  # 86 live combo columns: [context | ones | onehot]
WP = 128        # combo padded to full 128 columns (FWL-eligible LDWEIGHTS)
SKEW = 2        # MM2 trails MM1 by this many k-tiles

_CACHE = {}


def _build_graph():
    import concourse.bacc as bacc
    import concourse.tile as tile
    from concourse import mybir
    from concourse.masks import make_identity
    from concourse.hw_specs import get_activation_tables

    f32 = mybir.dt.float32
    f32r = mybir.dt.float32r
    f16 = mybir.dt.float16
    bf16 = mybir.dt.bfloat16
    AF = mybir.ActivationFunctionType

    nc = bacc.Bacc()
    qT_d = nc.declare_dram_parameter("qT", [D, Q], f16, isOutput=False)
    cT_d = nc.declare_dram_parameter("cT", [D, K], f16, isOutput=False)
    ctxb_d = nc.declare_dram_parameter("ctxb", [K, D], bf16, isOutput=False)
    lab_d = nc.declare_dram_parameter("lab", [128, KT], f32, isOutput=False)
    cv_d = nc.declare_dram_parameter("cv", [Q, D], f32, isOutput=True)
    lsc_d = nc.declare_dram_parameter("lsc", [Q, C1], f32, isOutput=True)

    act_sets = list(get_activation_tables("gen3").keys())
    NLE_ID = act_sets.index("natural_log_exp_and_others")

    with tile.TileContext(nc) as tc, ExitStack() as stk:
        singles = stk.enter_context(tc.tile_pool(name="singles", bufs=1))
        p_pool = stk.enter_context(tc.tile_pool(name="p_pool", bufs=4))
        acc_ps_cm = tc.tile_pool(name="acc_ps", bufs=1, space="PSUM")
        mm1_ps_cm = tc.tile_pool(name="mm1_ps", bufs=SKEW + 1, space="PSUM")
        acc_ps = acc_ps_cm.__enter__()
        mm1_ps = mm1_ps_cm.__enter__()

        warm_w = singles.tile([128, 128], f16)
        nc.vector.memset(warm_w[:], 0.25)

        # one combined exp+ln table load, placed before any ACTIVATE
        nc.scalar.add_instruction(mybir.InstLoadActFuncSet(
            name=nc.get_next_instruction_name(),
            act_func_set_id=NLE_ID, ins=[], outs=[]))

        # Input loads. MM1 operands are duplicated on partition halves
        # 0:64 (even k-tiles) and 64:128 (odd k-tiles). cT goes on the
        # sync HWDGE ring in chunks so the first k-tiles can start before
        # the full 2 MB lands; qT + labels go on the scalar HWDGE ring.
        qT_sb = singles.tile([128, Q], f16)
        for qh in (slice(0, Q // 2), slice(Q // 2, Q)):
            nc.scalar.dma_start(qT_sb[0:D, qh], qT_d[:, qh])
            nc.scalar.dma_start(qT_sb[D:2 * D, qh], qT_d[:, qh])
        lab_sb = singles.tile([128, KT], f32)
        nc.scalar.dma_start(lab_sb[:], lab_d[:])
        cT_sb = singles.tile([128, K], f16)
        bounds = [0, 128, 512, 2048, K]
        for g in range(len(bounds) - 1):
            cs = slice(bounds[g], bounds[g + 1])
            nc.sync.dma_start(cT_sb[0:D, cs], cT_d[:, cs])
            nc.sync.dma_start(cT_sb[D:2 * D, cs], cT_d[:, cs])

        ident = singles.tile([128, 128], f32)
        make_identity(nc, ident)
        iota = singles.tile([128, C1], f32)
        nc.gpsimd.iota(
            iota[:], pattern=[[1, C1]], base=0, channel_multiplier=0,
            allow_small_or_imprecise_dtypes=True,
        )

        # combo tensor [128, KT, WP] built once: ctx cols via one strided
        # DMA (gpsimd SWDGE ring, off the HWDGE rings), ones + zero-pad via
        # strided memsets, onehot via per-ktile is_equal
        combo = singles.tile([128, KT, WP], bf16)
        nc.scalar.dma_start(
            combo[:, :, 0:D], ctxb_d[:].rearrange("(n p) d -> p n d", p=128)
        )
        nc.vector.memset(combo[:, :, D:D + 1], 1.0)
        nc.vector.memset(combo[:, :, W:WP], 0.0)
        for kt in range(KT):
            nc.vector.tensor_scalar(
                combo[:, kt, D + 1:W], iota[:], lab_sb[:, kt:kt + 1], None,
                op0=mybir.AluOpType.is_equal,
            )

        acc = acc_ps.tile([128, Q], f32)
        # fill the input-DMA wait window with short matmuls so the PE HAM
        # throttle lifts before the real stream starts; acc is overwritten
        # by the first real MM2 (start=True)
        for i in range(50):
            nc.tensor.matmul(acc[:, 0:64], warm_w[:], warm_w[:, 0:64],
                             start=True, stop=True)

        def mm2(kt):
            for h in range(2):
                qs = slice(h * 512, (h + 1) * 512)
                nc.tensor.matmul(
                    acc[:, qs], combo[:, kt, :], pT_tiles[kt][:, qs],
                    start=(kt == 0), stop=(kt == KT - 1),
                )

        pT_tiles = {}
        mm1_tiles = {}
        HALod = [slice(0, D), slice(D, 2 * D)]
        for p in range(KT // 2 + 1):
            if p < KT // 2:
                kts = (2 * p, 2 * p + 1)
                for kt in kts:
                    mm1_tiles[kt] = mm1_ps.tile([128, Q], f32, name=f"mm1_{kt}", tag="mm1")
                # interleave halves so the two k-tiles run on different PE
                # row groups concurrently and LDWEIGHTS pulls ahead
                for h in range(2):
                    qs = slice(h * 512, (h + 1) * 512)
                    for j, kt in enumerate(kts):
                        ks = slice(kt * 128, (kt + 1) * 128)
                        nc.tensor.matmul(
                            mm1_tiles[kt][:, qs],
                            cT_sb[HALod[j], ks], qT_sb[HALod[j], qs],
                            start=True, stop=True,
                        )
                for kt in kts:
                    pT = p_pool.tile([128, Q], bf16)
                    nc.scalar.activation(pT[:], mm1_tiles[kt][:], AF.Exp)
                    pT_tiles[kt] = pT
            if p >= 1:
                mm2(2 * (p - 1))
                mm2(2 * (p - 1) + 1)

        acc_sb = singles.tile([W, Q], f32)
        for qt in range(QT):
            qs = slice(qt * 128, (qt + 1) * 128)
            nc.vector.tensor_copy(acc_sb[:, qs], acc[0:W, qs])

        # free the main-loop PSUM before allocating finale transpose banks
        mm1_ps_cm.__exit__(None, None, None)
        acc_ps_cm.__exit__(None, None, None)

        fin_ps = stk.enter_context(
            tc.tile_pool(name="fin_ps", bufs=4, space="PSUM"))
        small_pool = stk.enter_context(tc.tile_pool(name="small", bufs=4))
        fin_all = singles.tile([128, QT, W], f32)

        for qt in range(QT):
            qs = slice(qt * 128, (qt + 1) * 128)
            tp = fin_ps.tile([128, W], f32)
            nc.tensor.transpose(tp[:], acc_sb[:, qs], ident[0:W, 0:W])
            rec = small_pool.tile([128, 1], f32)
            nc.vector.reciprocal(rec[:], tp[:, D:D + 1])
            nc.vector.tensor_scalar_mul(
                fin_all[:, qt, 0:D], tp[:, 0:D], rec[:, 0:1])
            nc.vector.tensor_scalar_mul(
                fin_all[:, qt, D + 1:W], tp[:, D + 1:W], rec[:, 0:1])
            if qt == QT // 2 - 1:
                nc.sync.dma_start(
                    cv_d[0:Q // 2, :].rearrange("(n p) d -> p n d", p=128),
                    fin_all[:, 0:QT // 2, 0:D])
        nc.sync.dma_start(
            cv_d[Q // 2:Q, :].rearrange("(n p) d -> p n d", p=128),
            fin_all[:, QT // 2:QT, 0:D])
        H = QT // 2
        for hq in range(2):
            sl = slice(hq * H, (hq + 1) * H)
            nc.scalar.activation(
                fin_all[:, sl, D + 1:W], fin_all[:, sl, D + 1:W], AF.Ln)
            nc.sync.dma_start(
                lsc_d[hq * Q // 2:(hq + 1) * Q // 2, :].rearrange(
                    "(n p) c -> p n c", p=128),
                fin_all[:, sl, D + 1:W])

    nc.finalize()
    return nc


def _get_graph():
    if "nc" not in _CACHE:
        _CACHE["nc"] = _build_graph()
    return _CACHE["nc"]


def kernel(queries, context, context_labels, mask, num_classes,
           _profile=False):
    from concourse.bass_utils import run_bass_kernel_spmd

    queries = np.asarray(queries, dtype=np.float32)
    context = np.asarray(context, dtype=np.float32)
    labels = np.asarray(context_labels)
    mask = np.asarray(mask, dtype=np.float32)
    assert queries.shape == (B, Q, D) and context.shape == (B, K, D)
    assert int(num_classes) + 1 == C1

    nc = _get_graph()
    in_maps = []
    for b in range(B):
        cT = context[b].T * mask[b][None, :]  # fold mask into MM1 operand
        in_maps.append({
            "qT": np.ascontiguousarray(queries[b].T).astype(np.float16),
            "cT": np.ascontiguousarray(cT).astype(np.float16),
            "ctxb": np.asarray(context[b], dtype=ml_dtypes.bfloat16),
            "lab": np.ascontiguousarray(
                labels[b].reshape(KT, 128).T
            ).astype(np.float32),
        })

    res = run_bass_kernel_spmd(
        nc, in_maps, list(range(B)), trace=bool(_profile)
    )
    kernel._last_result = res
    cv = np.stack([res.results[i]["cv"] for i in range(B)])
    lsc = np.stack([res.results[i]["lsc"] for i in range(B)])
    return cv, lsc


# revision 24
# speedup vs baseline: 1.1839x; 1.1839x over previous
"""Trainium2 Bass kernel for nn_AttentionDist (attention + per-class logsumexp).

Math (per batch b):
    logits = queries @ context.T * mask          [Q, K]
    p      = softmax(logits, axis=-1)
    cv     = p @ context                          [Q, D]
    lsc[q,c] = logsumexp_{k: labels[k]==c} logp[q,k] = ln(S_c[q] / T[q])
  where P = exp(logits)  (no max-subtraction needed: |logits| <~ 50 fits f32),
        S_c = sum_{k in class c} P[q,k],  T = sum_k P[q,k].

Kernel strategy (data-parallel, core i <- batch i):
  MM1 (f16):   logitsT[kt]  = cT[:,kt].T @ qT          -> PSUM [128k, 1024q]
               even k-tiles use PE rows 0:64, odd k-tiles rows 64:128
               (operands duplicated across both partition halves) so
               consecutive LDWEIGHTS/MATMULs land on different row groups
               and overlap in the PE array.
  ACT:         P^T[kt]      = exp(logitsT[kt])         -> SBUF bf16
  MM2 (bf16):  acc         += [ctx|1|onehot|0][kt].T @ P^T[kt] -> PSUM [128, 1024]
               rows 0:64 = cv.T unnorm, 64 = T, 65:86 = S.T, 86:128 zeros
  Finale: PE-transpose acc to [q, 86] tiles, divide by T, ln for lsc.

The MM1->exp->MM2 chain is software-pipelined (MM2 trails by SKEW k-tiles)
so the PE never head-of-line blocks on the exp of the current k-tile. The
mask is folded in on the CPU by scaling cT columns.
"""

from contextlib import ExitStack

import numpy as np
import ml_dtypes

B, Q, K, D = 8, 1024, 4096, 64
C1 = 21
KT = K // 128   # 32 k-tiles
QT = Q // 128   # 8 q-tiles
W = D + 1 + C1  # 86 live combo columns: [context | ones | onehot]
WP = 128        # combo padded to full 128 columns (FWL-eligible LDWEIGHTS)
SKEW = 2        # MM2 trails MM1 by this many k-tiles

_CACHE = {}


def _build_graph():
    import concourse.bacc as bacc
    import concourse.tile as tile
    from concourse import mybir
    from concourse.masks import make_identity
    from concourse.hw_specs import get_activation_tables

    f32 = mybir.dt.float32
    f32r = mybir.dt.float32r
    f16 = mybir.dt.float16
    bf16 = mybir.dt.bfloat16
    AF = mybir.ActivationFunctionType

    nc = bacc.Bacc()
    qT_d = nc.declare_dram_parameter("qT", [D, Q], f16, isOutput=False)
    cT_d = nc.declare_dram_parameter("cT", [D, K], f16, isOutput=False)
    ctxb_d = nc.declare_dram_parameter("ctxb", [K, D], bf16, isOutput=False)
    lab_d = nc.declare_dram_parameter("lab", [128, KT], f32, isOutput=False)
    cv_d = nc.declare_dram_parameter("cv", [Q, D], f32, isOutput=True)
    lsc_d = nc.declare_dram_parameter("lsc", [Q, C1], f32, isOutput=True)

    act_sets = list(get_activation_tables("gen3").keys())
    NLE_ID = act_sets.index("natural_log_exp_and_others")

    with tile.TileContext(nc) as tc, ExitStack() as stk:
        singles = stk.enter_context(tc.tile_pool(name="singles", bufs=1))
        p_pool = stk.enter_context(tc.tile_pool(name="p_pool", bufs=4))
        acc_ps_cm = tc.tile_pool(name="acc_ps", bufs=1, space="PSUM")
        mm1_ps_cm = tc.tile_pool(name="mm1_ps", bufs=SKEW + 1, space="PSUM")
        acc_ps = acc_ps_cm.__enter__()
        mm1_ps = mm1_ps_cm.__enter__()

        warm_w = singles.tile([128, 128], f16)
        nc.vector.memset(warm_w[:], 0.25)

        # one combined exp+ln table load, placed before any ACTIVATE
        nc.scalar.add_instruction(mybir.InstLoadActFuncSet(
            name=nc.get_next_instruction_name(),
            act_func_set_id=NLE_ID, ins=[], outs=[]))

        # Input loads. MM1 operands are duplicated on partition halves
        # 0:64 (even k-tiles) and 64:128 (odd k-tiles). cT goes on the
        # sync HWDGE ring in chunks so the first k-tiles can start before
        # the full 2 MB lands; qT + labels go on the scalar HWDGE ring.
        qT_sb = singles.tile([128, Q], f16)
        for qh in (slice(0, Q // 2), slice(Q // 2, Q)):
            nc.scalar.dma_start(qT_sb[0:D, qh], qT_d[:, qh])
            nc.scalar.dma_start(qT_sb[D:2 * D, qh], qT_d[:, qh])
        lab_sb = singles.tile([128, KT], f32)
        nc.scalar.dma_start(lab_sb[:], lab_d[:])
        cT_sb = singles.tile([128, K], f16)
        bounds = [0, 128, 512, 2048, K]
        for g in range(len(bounds) - 1):
            cs = slice(bounds[g], bounds[g + 1])
            nc.sync.dma_start(cT_sb[0:D, cs], cT_d[:, cs])
            nc.sync.dma_start(cT_sb[D:2 * D, cs], cT_d[:, cs])

        ident = singles.tile([128, 128], f32)
        make_identity(nc, ident)
        iota = singles.tile([128, C1], f32)
        nc.gpsimd.iota(
            iota[:], pattern=[[1, C1]], base=0, channel_multiplier=0,
            allow_small_or_imprecise_dtypes=True,
        )

        # combo tensor [128, KT, WP] built once: ctx cols via one strided
        # DMA (gpsimd SWDGE ring, off the HWDGE rings), ones + zero-pad via
        # strided memsets, onehot via per-ktile is_equal
        combo = singles.tile([128, KT, WP], bf16)
        ctx_r = ctxb_d[:].rearrange("(n p) d -> p n d", p=128)
        for c0, c1 in ((0, 4), (4, 12), (12, 22), (22, KT)):
            nc.scalar.dma_start(combo[:, c0:c1, 0:D], ctx_r[:, c0:c1, :])
        nc.vector.memset(combo[:, :, D:D + 1], 1.0)
        nc.vector.memset(combo[:, :, W:WP], 0.0)
        for kt in range(KT):
            nc.vector.tensor_scalar(
                combo[:, kt, D + 1:W], iota[:], lab_sb[:, kt:kt + 1], None,
                op0=mybir.AluOpType.is_equal,
            )

        acc = acc_ps.tile([128, Q], f32)
        # fill the input-DMA wait window with short matmuls so the PE HAM
        # throttle lifts before the real stream starts; acc is overwritten
        # by the first real MM2 (start=True)
        for i in range(50):
            nc.tensor.matmul(acc[:, 0:64], warm_w[:], warm_w[:, 0:64],
                             start=True, stop=True)

        def mm2(kt):
            for h in range(2):
                qs = slice(h * 512, (h + 1) * 512)
                nc.tensor.matmul(
                    acc[:, qs], combo[:, kt, :], pT_tiles[kt][:, qs],
                    start=(kt == 0), stop=(kt == KT - 1),
                )

        pT_tiles = {}
        mm1_tiles = {}
        HALod = [slice(0, D), slice(D, 2 * D)]
        for p in range(KT // 2 + 1):
            if p < KT // 2:
                kts = (2 * p, 2 * p + 1)
                for kt in kts:
                    mm1_tiles[kt] = mm1_ps.tile([128, Q], f32, name=f"mm1_{kt}", tag="mm1")
                # interleave halves so the two k-tiles run on different PE
                # row groups concurrently and LDWEIGHTS pulls ahead
                for h in range(2):
                    qs = slice(h * 512, (h + 1) * 512)
                    for j, kt in enumerate(kts):
                        ks = slice(kt * 128, (kt + 1) * 128)
                        nc.tensor.matmul(
                            mm1_tiles[kt][:, qs],
                            cT_sb[HALod[j], ks], qT_sb[HALod[j], qs],
                            start=True, stop=True,
                        )
                for kt in kts:
                    pT = p_pool.tile([128, Q], bf16)
                    nc.scalar.activation(pT[:], mm1_tiles[kt][:], AF.Exp)
                    pT_tiles[kt] = pT
            if p >= 1:
                mm2(2 * (p - 1))
                mm2(2 * (p - 1) + 1)

        acc_sb = singles.tile([W, Q], f32)
        for qt in range(QT):
            qs = slice(qt * 128, (qt + 1) * 128)
            nc.vector.tensor_copy(acc_sb[:, qs], acc[0:W, qs])

        # free the main-loop PSUM before allocating finale transpose banks
        mm1_ps_cm.__exit__(None, None, None)
        acc_ps_cm.__exit__(None, None, None)

        fin_ps = stk.enter_context(
            tc.tile_pool(name="fin_ps", bufs=4, space="PSUM"))
        small_pool = stk.enter_context(tc.tile_pool(name="small", bufs=4))
        fin_all = singles.tile([128, QT, W], f32)

        for qt in range(QT):
            qs = slice(qt * 128, (qt + 1) * 128)
            tp = fin_ps.tile([128, W], f32)
            nc.tensor.transpose(tp[:], acc_sb[:, qs], ident[0:W, 0:W])
            rec = small_pool.tile([128, 1], f32)
            nc.vector.reciprocal(rec[:], tp[:, D:D + 1])
            nc.vector.tensor_scalar_mul(
                fin_all[:, qt, 0:D], tp[:, 0:D], rec[:, 0:1])
            nc.vector.tensor_scalar_mul(
                fin_all[:, qt, D + 1:W], tp[:, D + 1:W], rec[:, 0:1])
            if qt == QT // 2 - 1:
                nc.sync.dma_start(
                    cv_d[0:Q // 2, :].rearrange("(n p) d -> p n d", p=128),
                    fin_all[:, 0:QT // 2, 0:D])
        nc.sync.dma_start(
            cv_d[Q // 2:Q, :].rearrange("(n p) d -> p n d", p=128),
            fin_all[:, QT // 2:QT, 0:D])
        H = QT // 2
        for hq in range(2):
            sl = slice(hq * H, (hq + 1) * H)
            nc.scalar.activation(
                fin_all[:, sl, D + 1:W], fin_all[:, sl, D + 1:W], AF.Ln)
            nc.sync.dma_start(
                lsc_d[hq * Q // 2:(hq + 1) * Q // 2, :].rearrange(
                    "(n p) c -> p n c", p=128),
                fin_all[:, sl, D + 1:W])

    nc.finalize()
    return nc


def _get_graph():
    if "nc" not in _CACHE:
        _CACHE["nc"] = _build_graph()
    return _CACHE["nc"]


def kernel(queries, context, context_labels, mask, num_classes,
           _profile=False):
    from concourse.bass_utils import run_bass_kernel_spmd

    queries = np.asarray(queries, dtype=np.float32)
    context = np.asarray(context, dtype=np.float32)
    labels = np.asarray(context_labels)
    mask = np.asarray(mask, dtype=np.float32)
    assert queries.shape == (B, Q, D) and context.shape == (B, K, D)
    assert int(num_classes) + 1 == C1

    nc = _get_graph()
    in_maps = []
    for b in range(B):
        cT = context[b].T * mask[b][None, :]  # fold mask into MM1 operand
        in_maps.append({
            "qT": np.ascontiguousarray(queries[b].T).astype(np.float16),
            "cT": np.ascontiguousarray(cT).astype(np.float16),
            "ctxb": np.asarray(context[b], dtype=ml_dtypes.bfloat16),
            "lab": np.ascontiguousarray(
                labels[b].reshape(KT, 128).T
            ).astype(np.float32),
        })

    res = run_bass_kernel_spmd(
        nc, in_maps, list(range(B)), trace=bool(_profile)
    )
    kernel._last_result = res
    cv = np.stack([res.results[i]["cv"] for i in range(B)])
    lsc = np.stack([res.results[i]["lsc"] for i in range(B)])
    return cv, lsc
